# revision 1
# baseline (speedup 1.0000x reference)
import os, sys
import numpy as np

sys.path.insert(0, "/opt/trn_rl_repo")

import concourse.bass as bass
import concourse.bacc as bacc
import concourse.tile as tile
import concourse.mybir as mybir
from concourse.bass_utils import run_bass_kernel_spmd

F32 = mybir.dt.float32
F32R = mybir.dt.float32r
BF16 = mybir.dt.bfloat16
AF = mybir.ActivationFunctionType
ALU = mybir.AluOpType

NC = 8
B, C, H, W = 64, 128, 28, 28
BL = B // NC
HW = H * W
T = BL * HW                  # 6272
HEADS, D = 4, 32
E = 512
KV, L = 15, 225
EPS = 1e-5
NG = float(B * HW)
SCALE = D ** -0.5
NT, TCH = 14, 448
KC0, KC1 = 128, L - 128

last_result = None


def _f32r(ap):
    return ap.bitcast(F32R)


def _class_ranges(k):
    if k == 0:
        return (1, 2)
    if k == 1:
        return (0, 1, 2)
    return (0, 1)


def _host_prep(inputs):
    import ml_dtypes
    bf = ml_dtypes.bfloat16
    f = lambda a: np.ascontiguousarray(np.asarray(a), dtype=np.float32)
    inp = {k: np.asarray(v) for k, v in inputs.items()}
    h = {}

    def diag(wk, ntap, dt):
        ch = wk.shape[0]
        nch = ch // 128
        out = np.zeros((128, nch, ntap, 128), dtype=np.float32)
        for cc in range(nch):
            for t in range(ntap):
                out[np.arange(128), cc, t, np.arange(128)] = wk[cc * 128:(cc + 1) * 128, t]
        return np.ascontiguousarray(out.astype(dt))

    h["lpu_diag"] = diag(f(inp["lpu_w"]).reshape(C, 9), 9, bf).reshape(128, 9, 128)
    h["lpu_b"] = f(inp["lpu_b"]).reshape(C, 1)
    h["kdw_diag"] = diag(f(inp["kdw_w"]).reshape(C, 4), 4, bf).reshape(128, 4, 128)
    h["vdw_diag"] = diag(f(inp["vdw_w"]).reshape(C, 4), 4, bf).reshape(128, 4, 128)
    h["wqT"] = f(inp["wq"]).T.copy().astype(bf)
    h["wkT"] = f(inp["wk"]).T.copy().astype(bf)
    h["wvT"] = f(inp["wv"]).T.copy().astype(bf)
    h["woT"] = f(inp["wo"]).T.copy().astype(bf)
    h["bq"] = f(inp["bq"]).reshape(C, 1)
    h["bkp"] = (f(inp["bk"]) + f(inp["wk"]) @ f(inp["kdw_b"])).reshape(C, 1)
    bvp = f(inp["bv"]) + f(inp["wv"]) @ f(inp["vdw_b"])
    h["bop"] = (f(inp["bo"]) + f(inp["wo"]) @ bvp).reshape(C, 1)
    eb = np.exp(f(inp["attn_bias"]))[0].transpose(0, 2, 1)  # [4, 225, 784]
    ebp = np.zeros((128, 2, HEADS, HW), dtype=np.float32)
    ebp[:, 0] = eb[:, 0:128, :].transpose(1, 0, 2)
    ebp[:KC1, 1] = eb[:, 128:L, :].transpose(1, 0, 2)
    h["expb"] = np.ascontiguousarray(ebp.astype(bf))
    dww = f(inp["dw_w"]).reshape(E, 3, 3).copy()
    dww[:, 1, 1] += 1.0
    h["ffn_diag"] = diag(dww.reshape(E, 9), 9, np.float32)
    h["dw_b"] = f(inp["dw_b"]).reshape(4, 128).T.copy()
    psum9 = np.zeros((9, 4, 128), dtype=np.float32)
    for k in range(9):
        hr, wr = _class_ranges(k // 3), _class_ranges(k % 3)
        s = dww[:, hr, :][:, :, wr].sum(axis=(1, 2))
        psum9[k] = s.reshape(4, 128)
    h["psum9"] = psum9
    ind9 = np.zeros((9, H, W), dtype=np.float32)
    hc = np.full(H, 1); hc[0] = 0; hc[-1] = 2
    wc = np.full(W, 1); wc[0] = 0; wc[-1] = 2
    for i in range(H):
        for j in range(W):
            ind9[hc[i] * 3 + wc[j], i, j] = 1.0
    h["ind9"] = ind9.reshape(9, HW).astype(bf)
    h["c1wT"] = f(inp["c1_w"]).T.copy().astype(bf)
    h["c1_b"] = f(inp["c1_b"]).reshape(4, 128).T.copy()
    h["w2T"] = f(inp["c2_w"]).T.reshape(4, 128, 128).transpose(1, 0, 2).copy()
    h["bn1_g"] = f(inp["bn1_g"]).reshape(4, 128).T.copy()
    h["bn1_b"] = f(inp["bn1_b"]).reshape(4, 128).T.copy()
    h["bnr_g"] = f(inp["bnr_g"]).reshape(4, 128).T.copy()
    h["bnr_b"] = f(inp["bnr_b"]).reshape(4, 128).T.copy()
    h["bn2_g"] = f(inp["bn2_g"]).reshape(C, 1)
    h["bn2_b"] = f(inp["bn2_b"]).reshape(C, 1)
    ln_triv = (np.allclose(inp["ln1_g"], 1) and np.allclose(inp["ln1_b"], 0)
               and np.allclose(inp["ln2_g"], 1) and np.allclose(inp["ln2_b"], 0))
    h["_ln_triv"] = ln_triv
    if not ln_triv:
        h["ln1_g"] = f(inp["ln1_g"]).reshape(1, HW)
        h["ln1_b"] = f(inp["ln1_b"]).reshape(1, HW)
        h["ln2_g"] = f(inp["ln2_g"]).reshape(1, HW)
        h["ln2_b"] = f(inp["ln2_b"]).reshape(1, HW)
    return h


def _build(ln_triv):
    nc = bacc.Bacc(None, target_bir_lowering=False, num_devices=NC)
    dt = nc.dram_tensor
    xs = dt("xs", [BL, C, H, W], F32, kind="ExternalInput")
    out_t = dt("out", [BL, C, H, W], F32, kind="ExternalOutput")
    hin = {}
    specs = [
        ("lpu_diag", [128, 9, 128], BF16), ("lpu_b", [C, 1], F32),
        ("kdw_diag", [128, 4, 128], BF16), ("vdw_diag", [128, 4, 128], BF16),
        ("wqT", [C, C], BF16), ("wkT", [C, C], BF16), ("wvT", [C, C], BF16),
        ("woT", [C, C], BF16), ("bq", [C, 1], F32), ("bkp", [C, 1], F32),
        ("bop", [C, 1], F32), ("expb", [128, 2, HEADS, HW], BF16),
        ("ffn_diag", [128, 4, 9, 128], F32), ("dw_b", [128, 4], F32),
        ("psum9", [9, 4, 128], F32), ("ind9", [9, HW], BF16),
        ("c1wT", [C, E], BF16), ("c1_b", [128, 4], F32),
        ("w2T", [128, 4, 128], F32),
        ("bn1_g", [128, 4], F32), ("bn1_b", [128, 4], F32),
        ("bnr_g", [128, 4], F32), ("bnr_b", [128, 4], F32),
        ("bn2_g", [C, 1], F32), ("bn2_b", [C, 1], F32),
    ]
    if not ln_triv:
        specs += [(n, [1, HW], F32) for n in ["ln1_g", "ln1_b", "ln2_g", "ln2_b"]]
    for name, shape, d in specs:
        hin[name] = dt(name, shape, d, kind="ExternalInput")
    ar_in = {1: dt("ar1i", [128, 8], F32, kind="Internal"),
             2: dt("ar2i", [128, 8], F32, kind="Internal"),
             3: dt("ar3i", [128, 2], F32, kind="Internal")}
    ar_out = {1: dt("ar1o", [128, 8], F32, kind="Internal", addr_space="Shared"),
              2: dt("ar2o", [128, 8], F32, kind="Internal", addr_space="Shared"),
              3: dt("ar3o", [128, 2], F32, kind="Internal", addr_space="Shared")}
    c1_dram = dt("c1d", [128, 4], F32, kind="Internal")
    RG = [list(range(NC))]
    with tile.TileContext(nc) as tc:
        _emit(nc, tc, xs, out_t, hin, ar_in, ar_out, c1_dram, RG, ln_triv)
    if not nc.is_finalized():
        nc.finalize()
    return nc


def _emit(nc, tc, xs, out_t, hin, ar_in, ar_out, c1_dram, RG, ln_triv):
    from contextlib import ExitStack
    ctx = ExitStack()
    with ctx:
        big = ctx.enter_context(tc.tile_pool(name="big", bufs=2))
        bfp = ctx.enter_context(tc.tile_pool(name="bfp", bufs=1))
        shp = ctx.enter_context(tc.tile_pool(name="shp", bufs=2))
        cons = ctx.enter_context(tc.tile_pool(name="cons", bufs=1))
        small = ctx.enter_context(tc.tile_pool(name="small", bufs=1))
        epp = ctx.enter_context(tc.tile_pool(name="epp", bufs=1))
        ffp = ctx.enter_context(tc.tile_pool(name="ffp", bufs=1))
        psA = ctx.enter_context(tc.tile_pool(name="psA", bufs=4, space="PSUM"))
        psB = ctx.enter_context(tc.tile_pool(name="psB", bufs=2, space="PSUM"))
        psC = ctx.enter_context(tc.tile_pool(name="psC", bufs=2, space="PSUM"))

        def loadc(name):
            hh = hin[name]
            t = cons.tile(list(hh.shape), hh.dtype, tag=name)
            nc.gpsimd.dma_start(out=t, in_=hh[:])
            return t

        lpu_diag = loadc("lpu_diag"); lpu_b = loadc("lpu_b")
        kdw_diag = loadc("kdw_diag"); vdw_diag = loadc("vdw_diag")
        wqT = loadc("wqT"); wkT = loadc("wkT"); wvT = loadc("wvT"); woT = loadc("woT")
        bq = loadc("bq"); bkp = loadc("bkp"); bop = loadc("bop")
        dw_b = loadc("dw_b")
        psum9 = loadc("psum9"); ind9t = loadc("ind9")
        c1wT = loadc("c1wT"); c1_b = loadc("c1_b"); w2T = loadc("w2T")
        bn1_g = loadc("bn1_g"); bn1_b = loadc("bn1_b")
        bnr_g = loadc("bnr_g"); bnr_b = loadc("bnr_b")
        bn2_g = loadc("bn2_g"); bn2_b = loadc("bn2_b")
        ind9 = ind9t.rearrange("k (h w) -> k h w", h=H)
        lns = {}
        if not ln_triv:
            for nm in ["ln1_g", "ln1_b", "ln2_g", "ln2_b"]:
                t = cons.tile([128, HW], F32, tag=nm)
                nc.gpsimd.dma_start(out=t, in_=bass.AP(tensor=hin[nm], offset=0, ap=[[0, 128], [1, HW]]))
                lns[nm] = t
        epsT = small.tile([128, 1], F32, tag="epsT")
        nc.vector.memset(epsT, EPS)
        # pre-touch DMA-loaded consts on the engines that read them, so heavy
        # ops don't accumulate multiple DMA-queue sem waits (codegen limit)
        scrD = small.tile([128, 1], F32, tag="scrD")
        scrA = small.tile([128, 1], F32, tag="scrA")
        for t2 in (lpu_b, bq, bkp, bop, bn2_g, bn2_b):
            nc.vector.tensor_copy(out=scrD, in_=t2[:, 0:1])
        for t3 in (lpu_diag, kdw_diag, vdw_diag, w2T):
            nc.vector.tensor_copy(out=scrD, in_=t3[:, 0, 0:1])
        for t4 in (wqT, wkT, wvT, woT, c1wT):
            nc.vector.tensor_copy(out=scrD, in_=t4[:, 0:1])
        for t5 in (dw_b, c1_b, bn1_g, bn1_b, bnr_g, bnr_b):
            nc.vector.tensor_copy(out=scrD, in_=t5[:, 0:1])
        nc.vector.tensor_copy(out=scrD[0:9], in_=psum9[:, 0, 0:1])
        nc.vector.tensor_copy(out=scrD[0:9], in_=ind9t[:, 0:1])
        for t6 in lns.values():
            nc.vector.tensor_copy(out=scrD, in_=t6[:, 0:1])
        nc.scalar.mul(out=scrA, in_=c1_b[:, 0:1], mul=1.0)
        nc.scalar.mul(out=scrA, in_=dw_b[:, 0:1], mul=1.0)

        xsb = big.tile([128, BL, HW], F32, tag="big")
        nc.gpsimd.dma_start(out=xsb, in_=xs[:].rearrange("b c h w -> c b (h w)"))
        nc.vector.tensor_copy(out=scrD, in_=xsb[:, 0, 0:1])
        xbf = bfp.tile([128, BL, HW], BF16, tag="t12a")
        nc.gpsimd.tensor_copy(out=xbf, in_=xsb)
        xbf4 = xbf.rearrange("p b (h w) -> p b h w", h=H)

        # LPU dw (raw, bf16) + bias + residual -> x_lpu (f32)
        x_lpu = big.tile([128, BL, HW], F32, tag="big")
        xlp4 = x_lpu.rearrange("p b (h w) -> p b h w", h=H)
        xsb4 = xsb.rearrange("p b (h w) -> p b h w", h=H)

        def dw3x3(pt, dgrow, src4, base):
            first = True
            taps = [(1, 1)] + [(kh, kw) for kh in range(3) for kw in range(3) if (kh, kw) != (1, 1)]
            for n, (kh, kw) in enumerate(taps):
                r0 = max(0, 1 - kh - base); r1 = min(13, 28 - kh - base)
                c0 = max(0, 1 - kw); c1 = min(27, 28 - kw)
                if r1 < r0:
                    continue
                nc.tensor.matmul(
                    pt[:, r0:r1 + 1, c0:c1 + 1], dgrow[:, kh * 3 + kw, :],
                    src4[:, base + r0 + kh - 1: base + r1 + kh, c0 + kw - 1: c1 + kw],
                    start=first, stop=False, skip_group_check=True)
                first = False

        for b in range(BL):
            for half in range(2):
                base = 14 * half
                pt = psB.tile([128, 14, W], F32, tag="dwp")
                dw3x3(pt, lpu_diag, xbf4[:, b], base)
                nc.vector.scalar_tensor_tensor(
                    out=xlp4[:, b, base:base + 14, :], in0=pt, scalar=lpu_b,
                    in1=xsb4[:, b, base:base + 14, :], op0=ALU.add, op1=ALU.add)

        # LN over HW
        def layer_norm(src, gname, dst):
            sv = src.rearrange("p b (two q) -> p b two q", two=2)
            st = small.tile([128, BL, 2, 6], F32, tag="lnst")
            mv = small.tile([128, BL, 2], F32, tag="lnmv")
            sd = small.tile([128, BL, 1], F32, tag="lnsd")
            for b in range(BL):
                for g2 in range(2):
                    nc.vector.bn_stats(out=st[:, b, g2], in_=sv[:, b, g2])
                nc.vector.bn_aggr(out=mv[:, b], in_=st[:, b])
            nc.scalar.activation(out=sd, in_=mv[:, :, 1:2], func=AF.Sqrt, bias=epsT, scale=1.0)
            nc.vector.reciprocal(out=sd, in_=sd)
            for b in range(BL):
                nc.vector.tensor_scalar(
                    out=dst[:, b], in0=src[:, b], scalar1=mv[:, b, 0:1], scalar2=sd[:, b],
                    op0=ALU.subtract, op1=ALU.mult)
            if not ln_triv:
                g = lns[gname + "_g"]; bb = lns[gname + "_b"]
                for b in range(BL):
                    nc.vector.tensor_mul(out=dst[:, b], in0=dst[:, b], in1=g)
                    nc.vector.tensor_add(out=dst[:, b], in0=dst[:, b], in1=bb)

        xn = big.tile([128, BL, HW], F32, tag="big")
        layer_norm(x_lpu, "ln1", xn)
        xnbf = bfp.tile([128, BL, HW], BF16, tag="t12b")
        nc.gpsimd.tensor_copy(out=xnbf, in_=xn)
        xnbf6 = xnbf.rearrange("p b (hh t2 ww s2) -> p b hh t2 ww s2", t2=2, s2=2, hh=14)

        # Q projection (f32r) -> bf16
        qbf = bfp.tile([128, BL, HW], BF16, tag="qbf")
        xnbff = xnbf.rearrange("p b q -> p (b q)")
        qbff = qbf.rearrange("p b q -> p (b q)")
        for i in range(NT):
            pt = psA.tile([128, TCH], F32, tag="mm")
            nc.tensor.matmul(pt, wqT, xnbff[:, i * TCH:(i + 1) * TCH], start=True, stop=True)
            nc.vector.tensor_scalar(out=qbff[:, i * TCH:(i + 1) * TCH], in0=pt, scalar1=bq,
                                    scalar2=None, op0=ALU.add)
        # K/V strided 2x2 dw conv
        kxbf = bfp.tile([128, BL, L], BF16, tag="kxbf")
        vxbf = bfp.tile([128, BL, L], BF16, tag="vxbf")
        kx4 = kxbf.rearrange("p b (i j) -> p b i j", i=KV)
        vx4 = vxbf.rearrange("p b (i j) -> p b i j", i=KV)
        for b in range(BL):
            for dst4, dg in ((kx4, kdw_diag), (vx4, vdw_diag)):
                pt = psA.tile([128, KV, KV], F32, tag="mm")
                first = True
                for kh in range(2):
                    for kw in range(2):
                        i0 = 1 - kh; j0 = 1 - kw
                        hh0 = 0 if kh == 0 else 0
                        nc.tensor.matmul(
                            pt[:, i0:i0 + 14, j0:j0 + 14], dg[:, kh * 2 + kw, :],
                            xnbf6[:, b, 0:14, 1 - kh, 0:14, 1 - kw],
                            start=first, stop=False, skip_group_check=True)
                        first = False
                nc.scalar.copy(out=dst4[:, b], in_=pt)
        kbf = bfp.tile([128, BL, L], BF16, tag="kbf")
        kxf = kxbf.rearrange("p b l -> p (b l)")
        kbff = kbf.rearrange("p b l -> p (b l)")
        for i in range(4):
            pt = psA.tile([128, 450], F32, tag="mm")
            nc.tensor.matmul(pt, wkT, kxf[:, i * 450:(i + 1) * 450], start=True, stop=True)
            nc.vector.tensor_scalar(out=kbff[:, i * 450:(i + 1) * 450], in0=pt, scalar1=bkp,
                                    scalar2=None, op0=ALU.add)
        vaug = bfp.tile([128, BL, 2, HEADS, 64], BF16, tag="vaug")
        nc.vector.memset(vaug, 0.0)
        nc.vector.memset(vaug[:, :, :, :, 32:64], 1.0)
        for b in range(BL):
            for kc in range(2):
                ktM = KC0 if kc == 0 else KC1
                pt = psA.tile([128, 128], F32, tag="mm")
                nc.tensor.matmul(pt[0:ktM], vxbf[:, b, kc * 128: kc * 128 + ktM], wvT,
                                 start=True, stop=True)
                nc.scalar.copy(out=vaug[0:ktM, b, kc, :, 0:32],
                               in_=pt[0:ktM].rearrange("p (h d) -> p h d", h=HEADS))

        # attention
        o_sb = big.tile([128, BL, HW], F32, tag="big")
        rbc = bfp.tile([128, BL, HW], BF16, tag="t12a")
        for hd in range(HEADS):
            ebt = epp.tile([128, 2, HW], BF16, tag="ebt")
            nc.gpsimd.dma_start(out=ebt, in_=hin["expb"][:, :, hd, :])
            for b in range(BL):
                et = shp.tile([128, 2, HW], BF16, tag="et")
                for kc in range(2):
                    ktM = KC0 if kc == 0 else KC1
                    for qc in range(2):
                        pt = psA.tile([128, 392], F32, tag="mm")
                        nc.tensor.matmul(
                            pt[0:ktM],
                            kbf[hd * 32:(hd + 1) * 32, b, kc * 128: kc * 128 + ktM],
                            qbf[hd * 32:(hd + 1) * 32, b, qc * 392:(qc + 1) * 392],
                            start=True, stop=True, tile_position=(hd * 32, 0))
                        nc.scalar.activation(out=et[0:ktM, kc, qc * 392:(qc + 1) * 392],
                                             in_=pt[0:ktM], func=AF.Exp, scale=SCALE)
                nc.vector.tensor_mul(out=et, in0=et, in1=ebt)
                for qc in range(2):
                    pt = psC.tile([64, 392], F32, tag="av")
                    for kc in range(2):
                        ktM = KC0 if kc == 0 else KC1
                        nc.tensor.matmul(pt, vaug[0:ktM, b, kc, hd, :],
                                         et[0:ktM, kc, qc * 392:(qc + 1) * 392],
                                         start=(kc == 0), stop=(kc == 1))
                    nc.scalar.copy(out=o_sb[hd * 32:(hd + 1) * 32, b, qc * 392:(qc + 1) * 392],
                                   in_=pt[0:32])
                    with nc.allow_low_precision("softmax denominators in bf16"):
                        nc.vector.reciprocal(
                            out=rbc[hd * 32:(hd + 1) * 32, b, qc * 392:(qc + 1) * 392],
                            in_=pt[32:64])
        o_bf = bfp.tile([128, BL, HW], BF16, tag="t12b")
        nc.vector.tensor_mul(out=o_bf, in0=o_sb, in1=rbc)

        x_mhsa = o_sb
        of = o_bf.rearrange("p b q -> p (b q)")
        xmf = x_mhsa.rearrange("p b q -> p (b q)")
        xlf = x_lpu.rearrange("p b q -> p (b q)")
        for i in range(NT):
            pt = psA.tile([128, TCH], F32, tag="mm")
            nc.tensor.matmul(pt, woT, of[:, i * TCH:(i + 1) * TCH], start=True, stop=True)
            nc.vector.scalar_tensor_tensor(out=xmf[:, i * TCH:(i + 1) * TCH], in0=pt, scalar=bop,
                                           in1=xlf[:, i * TCH:(i + 1) * TCH], op0=ALU.add, op1=ALU.add)

        y = big.tile([128, BL, HW], F32, tag="big")
        layer_norm(x_mhsa, "ln2", y)

        def bn_reduce(src_r, nchunk, ar_i, ar_o):
            # src_r viewable [128, nchunk, NT, TCH]
            st = small.tile([128, nchunk, NT, 6], F32, tag="bnst")
            mv = small.tile([128, nchunk, 2], F32, tag="bnmv")
            for ecx in range(nchunk):
                for i in range(NT):
                    nc.vector.bn_stats(out=st[:, ecx, i], in_=src_r[:, ecx, i])
                nc.vector.bn_aggr(out=mv[:, ecx], in_=st[:, ecx])
            stats = small.tile([128, nchunk, 2], F32, tag="bnpack")
            m2 = small.tile([128, nchunk], F32, tag="bnm2")
            nc.vector.tensor_scalar(out=stats[:, :, 0:1], in0=mv[:, :, 0:1], scalar1=float(T),
                                    scalar2=None, op0=ALU.mult)
            nc.vector.tensor_mul(out=m2, in0=mv[:, :, 0], in1=mv[:, :, 0])
            nc.vector.tensor_add(out=m2, in0=m2, in1=mv[:, :, 1])
            nc.vector.tensor_scalar(out=stats[:, :, 1:2], in0=m2.rearrange("p (e o) -> p e o", o=1),
                                    scalar1=float(T), scalar2=None, op0=ALU.mult)
            nc.gpsimd.dma_start(out=ar_i[:], in_=stats.rearrange("p e two -> p (e two)"))
            nc.gpsimd.collective_compute("AllReduce", ALU.add, RG, ins=[ar_i[:]], outs=[ar_o[:]])
            g = small.tile([128, nchunk, 2], F32, tag="bngl")
            nc.gpsimd.dma_start(out=g.rearrange("p e two -> p (e two)"), in_=ar_o[:])
            return g

        def bn_affine(gs, nchunk, gt, bt):
            a = small.tile([128, nchunk], F32, tag="bna")
            cc = small.tile([128, nchunk], F32, tag="bnc")
            mean = small.tile([128, nchunk], F32, tag="bnmean")
            m2 = small.tile([128, nchunk], F32, tag="bnm2b")
            nc.vector.tensor_scalar(out=mean, in0=gs[:, :, 0], scalar1=1.0 / NG, scalar2=None, op0=ALU.mult)
            nc.vector.tensor_scalar(out=a, in0=gs[:, :, 1], scalar1=1.0 / NG, scalar2=None, op0=ALU.mult)
            nc.vector.tensor_mul(out=m2, in0=mean, in1=mean)
            nc.vector.tensor_sub(out=a, in0=a, in1=m2)
            nc.scalar.activation(out=a, in_=a, func=AF.Sqrt, bias=epsT, scale=1.0)
            nc.vector.reciprocal(out=a, in_=a)
            nc.vector.tensor_mul(out=a, in0=a, in1=gt)
            nc.vector.tensor_mul(out=cc, in0=mean, in1=a)
            nc.vector.scalar_tensor_tensor(out=cc, in0=cc, scalar=-1.0, in1=bt,
                                           op0=ALU.mult, op1=ALU.add)
            return a, cc

        # pw1 + gelu -> h1bf
        h1bf = bfp.tile([128, 4, BL, HW], BF16, tag="h1h2")
        h1f = h1bf.rearrange("p e b q -> p e (b q)")
        ybf = bfp.tile([128, BL, HW], BF16, tag="t12a")
        nc.gpsimd.tensor_copy(out=ybf, in_=y)
        ybff = ybf.rearrange("p b q -> p (b q)")
        for ec in range(4):
            for i in range(NT):
                pt = psA.tile([128, TCH], F32, tag="mm")
                nc.tensor.matmul(pt, c1wT[:, ec * 128:(ec + 1) * 128],
                                 ybff[:, i * TCH:(i + 1) * TCH], start=True, stop=True)
                nc.scalar.activation(out=h1f[:, ec, i * TCH:(i + 1) * TCH], in_=pt, func=AF.Gelu,
                                     bias=c1_b[:, ec:ec + 1], scale=1.0)
        gs1 = bn_reduce(h1f.rearrange("p e (n q) -> p e n q", q=TCH), 4, ar_in[1], ar_out[1])
        a1, c1 = bn_affine(gs1, 4, bn1_g, bn1_b)
        fds = bfp.tile([128, 4, 9, 128], BF16, tag="t12b")
        for ec in range(4):
            fdch = ffp.tile([128, 9, 128], F32, tag="fdch")
            nc.gpsimd.dma_start(out=fdch, in_=hin["ffn_diag"][:, ec])
            nc.vector.tensor_scalar(out=fds[:, ec].rearrange("p t c -> p (t c)"),
                                    in0=fdch.rearrange("p t c -> p (t c)"),
                                    scalar1=a1[:, ec:ec + 1], scalar2=None, op0=ALU.mult)
        nc.gpsimd.dma_start(out=c1_dram[:], in_=c1)
        c1row = small.tile([1, 4, 128], F32, tag="c1row")
        nc.gpsimd.dma_start(out=c1row, in_=bass.AP(tensor=c1_dram, offset=0, ap=[[0, 1], [1, 4], [4, 128]]))
        c1f = small.tile([9, 4, 128], F32, tag="c1f")
        nc.gpsimd.partition_broadcast(c1f, c1row)
        lh9 = small.tile([9, 4, 128], BF16, tag="lh9")
        nc.vector.tensor_mul(out=lh9, in0=psum9, in1=c1f)

        # FFN dw + border + gelu -> h2g
        h2g = h1bf
        h2g4 = h2g.rearrange("p e b (h w) -> p e b h w", h=H)
        h1b4 = h1bf.rearrange("p e b (h w) -> p e b h w", h=H)
        for ec in range(4):
            for b in range(BL):
                pts = []
                for half in range(2):
                    base = 14 * half
                    pt = psB.tile([128, 14, W], F32, tag="dwp")
                    dw3x3(pt, fds[:, ec], h1b4[:, ec, b], base)
                    nc.tensor.matmul(pt, lh9[:, ec], ind9[:, base:base + 14, :],
                                     start=False, stop=True, skip_group_check=True)
                    pts.append(pt)
                for half in range(2):
                    base = 14 * half
                    nc.scalar.activation(out=h2g4[:, ec, b, base:base + 14, :], in_=pts[half],
                                         func=AF.Gelu, bias=dw_b[:, ec:ec + 1], scale=1.0)
        h2f = h2g.rearrange("p e b q -> p e (b q)")
        gs2 = bn_reduce(h2f.rearrange("p e (n q) -> p e n q", q=TCH), 4, ar_in[2], ar_out[2])
        a2, c2 = bn_affine(gs2, 4, bnr_g, bnr_b)
        w2s = bfp.tile([128, 4, 128], BF16, tag="t12a")
        for kc in range(4):
            nc.vector.tensor_scalar(out=w2s[:, kc], in0=w2T[:, kc], scalar1=a2[:, kc:kc + 1],
                                    scalar2=None, op0=ALU.mult)
        ptb = psC.tile([128, 1], F32, tag="av")
        for kc in range(4):
            nc.tensor.matmul(ptb, w2T[:, kc], c2[:, kc:kc + 1], start=(kc == 0), stop=(kc == 3))
        biasc = small.tile([128, 1], F32, tag="biascS")
        nc.vector.tensor_copy(out=biasc, in_=ptb)

        # pw2 -> h3s
        h3s = big.tile([128, BL, HW], F32, tag="big")
        h3f = h3s.rearrange("p b q -> p (b q)")
        for i in range(NT):
            pt = psA.tile([128, TCH], F32, tag="mm")
            for kc in range(4):
                nc.tensor.matmul(pt, w2s[:, kc], h2f[:, kc, i * TCH:(i + 1) * TCH],
                                 start=(kc == 0), stop=(kc == 3))
            nc.vector.tensor_scalar(out=h3f[:, i * TCH:(i + 1) * TCH], in0=pt, scalar1=biasc,
                                    scalar2=None, op0=ALU.add)
        gs3 = bn_reduce(h3f.rearrange("p (o n q) -> p o n q", o=1, q=TCH), 1, ar_in[3], ar_out[3])
        a3, c3 = bn_affine(gs3, 1, bn2_g, bn2_b)

        nc.vector.tensor_scalar(out=h3f, in0=h3f, scalar1=a3, scalar2=c3,
                                op0=ALU.mult, op1=ALU.add)
        nc.vector.tensor_add(out=x_mhsa, in0=x_mhsa, in1=h3s)
        nc.sync.dma_start(out=out_t[:].rearrange("b c h w -> c b (h w)"), in_=x_mhsa)


_cached = None


def kernel(**inputs):
    global last_result, _cached
    hp = _host_prep(inputs)
    ln_triv = hp.pop("_ln_triv")
    if _cached is None or _cached[1] != ln_triv:
        _cached = (_build(ln_triv), ln_triv)
    nc = _cached[0]
    x = np.ascontiguousarray(np.asarray(inputs["x"], dtype=np.float32))
    in_maps = []
    for c in range(NC):
        m = dict(hp)
        m["xs"] = np.ascontiguousarray(x[c * BL:(c + 1) * BL])
        in_maps.append(m)
    trace = os.environ.get("KERNEL_TRACE", "0") == "1"
    res = run_bass_kernel_spmd(nc, in_maps, core_ids=list(range(NC)), trace=trace)
    last_result = res
    return np.concatenate([r["out"] for r in res.results], axis=0)



# revision 18
# speedup vs baseline: 1.5153x; 1.5153x over previous
import os, sys
import numpy as np

sys.path.insert(0, "/opt/trn_rl_repo")

import concourse.bass as bass
import concourse.bacc as bacc
import concourse.tile as tile
import concourse.mybir as mybir
from concourse.bass_utils import run_bass_kernel_spmd

F32 = mybir.dt.float32
BF16 = mybir.dt.bfloat16
AF = mybir.ActivationFunctionType
ALU = mybir.AluOpType

NC = 8
B, C, H, W = 64, 128, 28, 28
BL = B // NC
HW = H * W
T = BL * HW                  # 6272
HEADS, D = 4, 32
E = 512
KV, L = 15, 225
EPS = 1e-5
NG = float(B * HW)
SCALE = D ** -0.5
NT, TCH = 14, 448
KC0, KC1 = 128, L - 128
NEG = -1e30

last_result = None


def _class_ranges(k):
    if k == 0:
        return (1, 2)
    if k == 1:
        return (0, 1, 2)
    return (0, 1)


def _host_prep(inputs):
    import ml_dtypes
    bf = ml_dtypes.bfloat16
    f = lambda a: np.ascontiguousarray(np.asarray(a), dtype=np.float32)
    inp = {k: np.asarray(v) for k, v in inputs.items()}
    h = {}

    def diag(wk, ntap, dt):
        ch = wk.shape[0]
        nch = ch // 128
        out = np.zeros((128, nch, ntap, 128), dtype=np.float32)
        for cc in range(nch):
            for t in range(ntap):
                out[np.arange(128), cc, t, np.arange(128)] = wk[cc * 128:(cc + 1) * 128, t]
        return np.ascontiguousarray(out.astype(dt))

    h["lpu_diag"] = diag(f(inp["lpu_w"]).reshape(C, 9), 9, bf).reshape(128, 9, 128)
    h["lpu_b"] = f(inp["lpu_b"]).reshape(C, 1)
    h["kdw_diag"] = diag(f(inp["kdw_w"]).reshape(C, 4), 4, bf).reshape(128, 4, 128)
    h["vdw_diag"] = diag(f(inp["vdw_w"]).reshape(C, 4), 4, bf).reshape(128, 4, 128)
    h["wqT"] = f(inp["wq"]).T.copy().astype(bf)
    h["wkT"] = f(inp["wk"]).T.copy().astype(bf)
    h["wvT"] = f(inp["wv"]).T.copy().astype(bf)
    h["woT"] = f(inp["wo"]).T.copy().astype(bf)
    h["bq"] = f(inp["bq"]).reshape(C, 1)
    h["bkp"] = (f(inp["bk"]) + f(inp["wk"]) @ f(inp["kdw_b"])).reshape(C, 1)
    bvp = f(inp["bv"]) + f(inp["wv"]) @ f(inp["vdw_b"])
    h["bop"] = (f(inp["bo"]) + f(inp["wo"]) @ bvp).reshape(C, 1)
    # attention bias in log domain, pre-divided by SCALE so that
    # exp(SCALE*(qk + bias/SCALE)) = exp(SCALE*qk + bias).
    # layout [128 key-partitions, 2 kc, 4 heads, HW]; rows beyond the
    # valid key count get a huge negative (exp -> 0) so the AV matmul can
    # contract over the full 128 partitions.
    ab = f(inp["attn_bias"])[0].transpose(0, 2, 1) / SCALE  # [4, 225, 784]
    lb = np.full((128, 2, HEADS, HW), NEG, dtype=np.float32)
    lb[:, 0] = ab[:, 0:128, :].transpose(1, 0, 2)
    lb[:KC1, 1] = ab[:, 128:L, :].transpose(1, 0, 2)
    h["logeb"] = np.ascontiguousarray(lb.astype(bf))
    h["ident"] = np.eye(128, dtype=np.float32).astype(bf)
    dww = f(inp["dw_w"]).reshape(E, 3, 3).copy()
    dww[:, 1, 1] += 1.0
    h["ffn_diag"] = diag(dww.reshape(E, 9), 9, bf)
    h["dw_b"] = f(inp["dw_b"]).reshape(4, 128).T.copy()
    psum9 = np.zeros((9, 4, 128), dtype=np.float32)
    for k in range(9):
        hr, wr = _class_ranges(k // 3), _class_ranges(k % 3)
        s = dww[:, hr, :][:, :, wr].sum(axis=(1, 2))
        psum9[k] = s.reshape(4, 128)
    h["psum9"] = psum9
    ind9 = np.zeros((9, H, W), dtype=np.float32)
    hc = np.full(H, 1); hc[0] = 0; hc[-1] = 2
    wc = np.full(W, 1); wc[0] = 0; wc[-1] = 2
    for i in range(H):
        for j in range(W):
            ind9[hc[i] * 3 + wc[j], i, j] = 1.0
    h["ind9"] = ind9.reshape(9, HW).astype(bf)
    h["c1wT"] = f(inp["c1_w"]).T.copy().astype(bf)
    h["c1_b"] = f(inp["c1_b"]).reshape(4, 128).T.copy()
    h["w2T"] = f(inp["c2_w"]).T.reshape(4, 128, 128).transpose(1, 0, 2).copy()
    h["bn1_g"] = f(inp["bn1_g"]).reshape(4, 128).T.copy()
    h["bn1_b"] = f(inp["bn1_b"]).reshape(4, 128).T.copy()
    h["bnr_g"] = f(inp["bnr_g"]).reshape(4, 128).T.copy()
    h["bnr_b"] = f(inp["bnr_b"]).reshape(4, 128).T.copy()
    h["bn2_g"] = f(inp["bn2_g"]).reshape(C, 1)
    h["bn2_b"] = f(inp["bn2_b"]).reshape(C, 1)
    ln_triv = (np.allclose(inp["ln1_g"], 1) and np.allclose(inp["ln1_b"], 0)
               and np.allclose(inp["ln2_g"], 1) and np.allclose(inp["ln2_b"], 0))
    h["_ln_triv"] = ln_triv
    if not ln_triv:
        h["ln1_g"] = f(inp["ln1_g"]).reshape(1, HW)
        h["ln1_b"] = f(inp["ln1_b"]).reshape(1, HW)
        h["ln2_g"] = f(inp["ln2_g"]).reshape(1, HW)
        h["ln2_b"] = f(inp["ln2_b"]).reshape(1, HW)
    return h


def _build(ln_triv):
    nc = bacc.Bacc(None, target_bir_lowering=False, num_devices=NC)
    dt = nc.dram_tensor
    xs = dt("xs", [BL, C, H, W], F32, kind="ExternalInput")
    out_t = dt("out", [BL, C, H, W], F32, kind="ExternalOutput")
    hin = {}
    specs = [
        ("lpu_diag", [128, 9, 128], BF16), ("lpu_b", [C, 1], F32),
        ("kdw_diag", [128, 4, 128], BF16), ("vdw_diag", [128, 4, 128], BF16),
        ("wqT", [C, C], BF16), ("wkT", [C, C], BF16), ("wvT", [C, C], BF16),
        ("woT", [C, C], BF16), ("bq", [C, 1], F32), ("bkp", [C, 1], F32),
        ("bop", [C, 1], F32), ("logeb", [128, 2, HEADS, HW], BF16),
        ("ident", [128, 128], BF16),
        ("ffn_diag", [128, 4, 9, 128], BF16), ("dw_b", [128, 4], F32),
        ("psum9", [9, 4, 128], F32), ("ind9", [9, HW], BF16),
        ("c1wT", [C, E], BF16), ("c1_b", [128, 4], F32),
        ("w2T", [128, 4, 128], F32),
        ("bn1_g", [128, 4], F32), ("bn1_b", [128, 4], F32),
        ("bnr_g", [128, 4], F32), ("bnr_b", [128, 4], F32),
        ("bn2_g", [C, 1], F32), ("bn2_b", [C, 1], F32),
    ]
    if not ln_triv:
        specs += [(n, [1, HW], F32) for n in ["ln1_g", "ln1_b", "ln2_g", "ln2_b"]]
    for name, shape, d in specs:
        hin[name] = dt(name, shape, d, kind="ExternalInput")
    ar_in = {1: dt("ar1i", [128, 8], F32, kind="Internal"),
             2: dt("ar2i", [128, 8], F32, kind="Internal"),
             3: dt("ar3i", [128, 2], F32, kind="Internal")}
    ar_out = {1: dt("ar1o", [128, 8], F32, kind="Internal", addr_space="Shared"),
              2: dt("ar2o", [128, 8], F32, kind="Internal", addr_space="Shared"),
              3: dt("ar3o", [128, 2], F32, kind="Internal", addr_space="Shared")}
    c1_dram = dt("c1d", [128, 4], F32, kind="Internal")
    RG = [list(range(NC))]
    with tile.TileContext(nc) as tc:
        _emit(nc, tc, xs, out_t, hin, ar_in, ar_out, c1_dram, RG, ln_triv)
    if not nc.is_finalized():
        nc.finalize()
    return nc


def _emit(nc, tc, xs, out_t, hin, ar_in, ar_out, c1_dram, RG, ln_triv):
    from contextlib import ExitStack
    ctx = ExitStack()
    with ctx:
        big = ctx.enter_context(tc.tile_pool(name="big", bufs=2))
        bfp = ctx.enter_context(tc.tile_pool(name="bfp", bufs=1))
        cons = ctx.enter_context(tc.tile_pool(name="cons", bufs=1))
        small = ctx.enter_context(tc.tile_pool(name="small", bufs=1))
        ps = ctx.enter_context(tc.tile_pool(name="ps", bufs=1, space="PSUM"))

        def loadc(name):
            hh = hin[name]
            t = cons.tile(list(hh.shape), hh.dtype, tag=name, name=name)
            nc.gpsimd.dma_start(out=t, in_=hh[:])
            return t

        lpu_diag = loadc("lpu_diag"); lpu_b = loadc("lpu_b")
        kdw_diag = loadc("kdw_diag"); vdw_diag = loadc("vdw_diag")
        wqT = loadc("wqT"); wkT = loadc("wkT"); wvT = loadc("wvT"); woT = loadc("woT")
        bq = loadc("bq"); bkp = loadc("bkp"); bop = loadc("bop")
        # logeb shares the (disjoint-lifetime) h1/h2 slot to save SBUF
        logeb = bfp.tile([128, 2, HEADS, HW], BF16, tag="h1h2", name="logeb")
        nc.gpsimd.dma_start(out=logeb, in_=hin["logeb"][:])
        ident = loadc("ident")
        ffn_diag = loadc("ffn_diag"); dw_b = loadc("dw_b")
        psum9 = loadc("psum9"); ind9t = loadc("ind9")
        c1wT = loadc("c1wT"); c1_b = loadc("c1_b"); w2T = loadc("w2T")
        bn1_g = loadc("bn1_g"); bn1_b = loadc("bn1_b")
        bnr_g = loadc("bnr_g"); bnr_b = loadc("bnr_b")
        bn2_g = loadc("bn2_g"); bn2_b = loadc("bn2_b")
        ind9 = ind9t.rearrange("k (h w) -> k h w", h=H)
        lns = {}
        if not ln_triv:
            for nm in ["ln1_g", "ln1_b", "ln2_g", "ln2_b"]:
                t = cons.tile([128, HW], F32, tag=nm, name=nm)
                nc.gpsimd.dma_start(out=t, in_=bass.AP(tensor=hin[nm], offset=0, ap=[[0, 128], [1, HW]]))
                lns[nm] = t
        epsT = small.tile([128, 1], F32, tag="epsT")
        nc.vector.memset(epsT, EPS)

        # input load: per-sample DMAs on the sync queue (parallel with the
        # const loads on the gpsimd queue), bf16 cast on vector per sample
        xsb = big.tile([128, BL, HW], F32, tag="big")
        for b in range(BL):
            nc.sync.dma_start(out=xsb[:, b], in_=xs[b].rearrange("c h w -> c (h w)"))
        xbf = bfp.tile([128, BL, HW], BF16, tag="t1", bufs=2)
        for b in range(BL):
            nc.vector.tensor_copy(out=xbf[:, b], in_=xsb[:, b])
        xbf4 = xbf.rearrange("p b (h w) -> p b h w", h=H)

        # LPU dw (raw, bf16) + bias + residual -> x_lpu (f32)
        x_lpu = big.tile([128, BL, HW], F32, tag="big")
        xlp4 = x_lpu.rearrange("p b (h w) -> p b h w", h=H)
        xsb4 = xsb.rearrange("p b (h w) -> p b h w", h=H)

        def dw3x3(pt, dgrow, src4, base):
            first = True
            taps = [(1, 1)] + [(kh, kw) for kh in range(3) for kw in range(3) if (kh, kw) != (1, 1)]
            for n, (kh, kw) in enumerate(taps):
                r0 = max(0, 1 - kh - base); r1 = min(13, 28 - kh - base)
                c0 = max(0, 1 - kw); c1 = min(27, 28 - kw)
                if r1 < r0:
                    continue
                nc.tensor.matmul(
                    pt[:, r0:r1 + 1, c0:c1 + 1], dgrow[:, kh * 3 + kw, :],
                    src4[:, base + r0 + kh - 1: base + r1 + kh, c0 + kw - 1: c1 + kw],
                    start=first, stop=False, skip_group_check=True)
                first = False

        for b in range(BL):
            pt2 = ps.tile([128, 2, 512], F32, tag="A" if b % 2 == 0 else "Bk")
            for half in range(2):
                base = 14 * half
                pth = pt2[:, half, 0:392].rearrange("p (r c) -> p r c", c=W)
                dw3x3(pth, lpu_diag, xbf4[:, b], base)
            for half in range(2):
                base = 14 * half
                pth = pt2[:, half, 0:392].rearrange("p (r c) -> p r c", c=W)
                nc.vector.scalar_tensor_tensor(
                    out=xlp4[:, b, base:base + 14, :], in0=pth, scalar=lpu_b,
                    in1=xsb4[:, b, base:base + 14, :], op0=ALU.add, op1=ALU.add)

        # LN over HW; normalized output written directly as bf16
        def layer_norm(src, gname, dst_bf):
            sv = src.rearrange("p b (two q) -> p b two q", two=2)
            st = small.tile([128, BL, 2, 6], F32, tag="lnst")
            mv = small.tile([128, BL, 2], F32, tag="lnmv")
            sd = small.tile([128, BL, 1], F32, tag="lnsd")
            for b in range(BL):
                for g2 in range(2):
                    nc.vector.bn_stats(out=st[:, b, g2], in_=sv[:, b, g2])
                nc.vector.bn_aggr(out=mv[:, b], in_=st[:, b])
            nc.scalar.activation(out=sd, in_=mv[:, :, 1:2], func=AF.Sqrt, bias=epsT, scale=1.0)
            nc.vector.reciprocal(out=sd, in_=sd)
            for b in range(BL):
                nc.vector.tensor_scalar(
                    out=dst_bf[:, b], in0=src[:, b], scalar1=mv[:, b, 0:1], scalar2=sd[:, b],
                    op0=ALU.subtract, op1=ALU.mult)
            if not ln_triv:
                g = lns[gname + "_g"]; bb = lns[gname + "_b"]
                for b in range(BL):
                    nc.vector.tensor_mul(out=dst_bf[:, b], in0=dst_bf[:, b], in1=g)
                    nc.vector.tensor_add(out=dst_bf[:, b], in0=dst_bf[:, b], in1=bb)

        xnbf = bfp.tile([128, BL, HW], BF16, tag="t2")
        layer_norm(x_lpu, "ln1", xnbf)
        xnbf6 = xnbf.rearrange("p b (hh t2 ww s2) -> p b hh t2 ww s2", t2=2, s2=2, hh=14)

        # Q projection -> bf16
        qbf = bfp.tile([128, BL, HW], BF16, tag="t3")
        xnbff = xnbf.rearrange("p b q -> p (b q)")
        qbff = qbf.rearrange("p b q -> p (b q)")
        for i in range(NT):
            pt = ps.tile([128, 2, 512], F32, tag="A" if i % 2 == 0 else "Bk")
            nc.tensor.matmul(pt[:, 0, 0:448], wqT, xnbff[:, i * TCH:(i + 1) * TCH], start=True, stop=True)
            nc.vector.tensor_scalar(out=qbff[:, i * TCH:(i + 1) * TCH], in0=pt[:, 0, 0:448], scalar1=bq,
                                    scalar2=None, op0=ALU.add)
        # K/V strided 2x2 dw conv
        kxbf = bfp.tile([128, BL, L], BF16, tag="kxbf")
        vxbf = bfp.tile([128, BL, L], BF16, tag="vxbf")
        kx4 = kxbf.rearrange("p b (i j) -> p b i j", i=KV)
        vx4 = vxbf.rearrange("p b (i j) -> p b i j", i=KV)
        for b in range(BL):
            pt2 = ps.tile([128, 2, 512], F32, tag="A" if b % 2 == 0 else "Bk")
            for ci, dg in ((0, kdw_diag), (1, vdw_diag)):
                pt = pt2[:, ci, 0:KV * KV].rearrange("p (i j) -> p i j", i=KV)
                first = True
                for kh in range(2):
                    for kw in range(2):
                        i0 = 1 - kh; j0 = 1 - kw
                        nc.tensor.matmul(
                            pt[:, i0:i0 + 14, j0:j0 + 14], dg[:, kh * 2 + kw, :],
                            xnbf6[:, b, 0:14, 1 - kh, 0:14, 1 - kw],
                            start=first, stop=False, skip_group_check=True)
                        first = False
            nc.scalar.copy(out=kx4[:, b], in_=pt2[:, 0, 0:KV * KV].rearrange("p (i j) -> p i j", i=KV))
            nc.scalar.copy(out=vx4[:, b], in_=pt2[:, 1, 0:KV * KV].rearrange("p (i j) -> p i j", i=KV))
        kbf = bfp.tile([128, BL, L], BF16, tag="kbf")
        kxf = kxbf.rearrange("p b l -> p (b l)")
        kbff = kbf.rearrange("p b l -> p (b l)")
        for i in range(4):
            pt = ps.tile([128, 2, 512], F32, tag="A" if i % 2 == 0 else "Bk")
            nc.tensor.matmul(pt[:, 0, 0:450], wkT, kxf[:, i * 450:(i + 1) * 450], start=True, stop=True)
            nc.vector.tensor_scalar(out=kbff[:, i * 450:(i + 1) * 450], in0=pt[:, 0, 0:450], scalar1=bkp,
                                    scalar2=None, op0=ALU.add)
        # V: vaug[k, b, kc, 128] = projected v for all 4 heads
        vaug = bfp.tile([128, BL, 2, 128], BF16, tag="vaug")
        nc.vector.memset(vaug.rearrange("p b kc x -> p (b kc x)"), 0.0)
        ones32 = small.tile([128, 32], BF16, tag="ones32")
        nc.vector.memset(ones32, 1.0)
        for b in range(BL):
            for kc in range(2):
                ktM = KC0 if kc == 0 else KC1
                pt = ps.tile([128, 2, 512], F32, tag="A" if kc == 0 else "Bk")
                nc.tensor.matmul(pt[0:ktM, 0, 0:128], vxbf[:, b, kc * 128: kc * 128 + ktM], wvT,
                                 start=True, stop=True)
                nc.scalar.copy(out=vaug[0:ktM, b, kc], in_=pt[0:ktM, 0, 0:128])

        # ---------------- attention ----------------
        o_bf = bfp.tile([128, BL, HW], BF16, tag="t2")
        for b in range(BL):
            et = bfp.tile([128, 2, HEADS, HW], BF16, tag="t1", bufs=2)
            for kc in range(2):
                ktM = KC0 if kc == 0 else KC1
                for qc in range(2):
                    pt = ps.tile([128, 2, 512], F32, tag="A" if qc == 0 else "Bk")
                    for pr in range(2):
                        if pr == 1:
                            pt = ps.tile([128, 2, 512], F32, tag="A2" if qc == 0 else "B2")
                        for j in range(2):
                            hd = pr * 2 + j
                            # bias add (full 128 rows; pads invalid keys to -1e30)
                            nc.tensor.matmul(
                                pt[:, j, 0:392], ident,
                                logeb[:, kc, hd, qc * 392:(qc + 1) * 392],
                                start=True, stop=False, skip_group_check=True)
                        for j in range(2):
                            hd = pr * 2 + j
                            # scores, 4 heads on distinct PE row groups
                            nc.tensor.matmul(
                                pt[0:ktM, j, 0:392],
                                kbf[hd * 32:(hd + 1) * 32, b, kc * 128: kc * 128 + ktM],
                                qbf[hd * 32:(hd + 1) * 32, b, qc * 392:(qc + 1) * 392],
                                start=False, stop=True, skip_group_check=True,
                                tile_position=(hd * 32, 0))
                        nc.scalar.activation(
                            out=et[:, kc, 2 * pr:2 * pr + 2, qc * 392:(qc + 1) * 392],
                            in_=pt[:, :, 0:392], func=AF.Exp, scale=SCALE)
            # AV: o for all 4 heads on col strips of one bank-pair; the
            # softmax denominators on a second bank-pair at the SAME
            # partitions, so recip + divide need no partition shuffles
            pto = ps.tile([128, 2, 512], F32, tag="A")
            ptd = ps.tile([128, 2, 512], F32, tag="Bk")
            for qc in range(2):
                for kc in range(2):
                    for hd in range(HEADS):
                        nc.tensor.matmul(
                            pto[hd * 32:(hd + 1) * 32, qc, 0:392],
                            vaug[:, b, kc, hd * 32:(hd + 1) * 32],
                            et[:, kc, hd, qc * 392:(qc + 1) * 392],
                            start=(kc == 0), stop=(kc == 1),
                            tile_position=(0, hd * 32), skip_group_check=True)
                    for hd in range(HEADS):
                        nc.tensor.matmul(
                            ptd[hd * 32:(hd + 1) * 32, qc, 0:392],
                            ones32, et[:, kc, hd, qc * 392:(qc + 1) * 392],
                            start=(kc == 0), stop=(kc == 1),
                            tile_position=(0, hd * 32), skip_group_check=True)
            rf = small.tile([128, 2, 392], F32, tag="rf")
            nc.vector.reciprocal_approx_fast(out=rf, in_=ptd[:, :, 0:392])
            nc.vector.tensor_mul(
                out=o_bf[:, b].rearrange("p (a q) -> p a q", a=2),
                in0=pto[:, :, 0:392], in1=rf)

        # O projection + residual -> x_mhsa (f32)
        x_mhsa = big.tile([128, BL, HW], F32, tag="big")
        of = o_bf.rearrange("p b q -> p (b q)")
        xmf = x_mhsa.rearrange("p b q -> p (b q)")
        xlf = x_lpu.rearrange("p b q -> p (b q)")
        for i in range(NT):
            pt = ps.tile([128, 2, 512], F32, tag="A" if i % 2 == 0 else "Bk")
            nc.tensor.matmul(pt[:, 0, 0:448], woT, of[:, i * TCH:(i + 1) * TCH], start=True, stop=True)
            nc.vector.scalar_tensor_tensor(out=xmf[:, i * TCH:(i + 1) * TCH], in0=pt[:, 0, 0:448],
                                           scalar=bop, in1=xlf[:, i * TCH:(i + 1) * TCH],
                                           op0=ALU.add, op1=ALU.add)

        ybf = bfp.tile([128, BL, HW], BF16, tag="t3")
        layer_norm(x_mhsa, "ln2", ybf)

        def bn_pack_reduce(mv, nchunk, ar_i, ar_o):
            # mv [128, nchunk, 2] local mean/var -> allreduced sums
            stats = small.tile([128, nchunk, 2], F32, tag="bnpack")
            m2 = small.tile([128, nchunk], F32, tag="bnm2")
            nc.vector.tensor_scalar(out=stats[:, :, 0:1], in0=mv[:, :, 0:1], scalar1=float(T),
                                    scalar2=None, op0=ALU.mult)
            nc.vector.tensor_mul(out=m2, in0=mv[:, :, 0], in1=mv[:, :, 0])
            nc.vector.tensor_add(out=m2, in0=m2, in1=mv[:, :, 1])
            nc.vector.tensor_scalar(out=stats[:, :, 1:2], in0=m2.rearrange("p (e o) -> p e o", o=1),
                                    scalar1=float(T), scalar2=None, op0=ALU.mult)
            nc.gpsimd.dma_start(out=ar_i[:], in_=stats.rearrange("p e two -> p (e two)"))
            nc.gpsimd.collective_compute("AllReduce", ALU.add, RG, ins=[ar_i[:]], outs=[ar_o[:]])
            g = small.tile([128, nchunk, 2], F32, tag="bngl")
            nc.gpsimd.dma_start(out=g.rearrange("p e two -> p (e two)"), in_=ar_o[:])
            return g

        def bn_affine(gs, nchunk, gt, bt):
            a = small.tile([128, nchunk], F32, tag="bna", bufs=3)
            cc = small.tile([128, nchunk], F32, tag="bnc", bufs=3)
            mean = small.tile([128, nchunk], F32, tag="bnmean")
            m2 = small.tile([128, nchunk], F32, tag="bnm2b")
            nc.vector.tensor_scalar(out=mean, in0=gs[:, :, 0], scalar1=1.0 / NG, scalar2=None, op0=ALU.mult)
            nc.vector.tensor_scalar(out=a, in0=gs[:, :, 1], scalar1=1.0 / NG, scalar2=None, op0=ALU.mult)
            nc.vector.tensor_mul(out=m2, in0=mean, in1=mean)
            nc.vector.tensor_sub(out=a, in0=a, in1=m2)
            nc.scalar.activation(out=a, in_=a, func=AF.Sqrt, bias=epsT, scale=1.0)
            nc.vector.reciprocal(out=a, in_=a)
            nc.vector.tensor_mul(out=a, in0=a, in1=gt)
            nc.vector.tensor_mul(out=cc, in0=mean, in1=a)
            nc.vector.scalar_tensor_tensor(out=cc, in0=cc, scalar=-1.0, in1=bt,
                                           op0=ALU.mult, op1=ALU.add)
            return a, cc

        # pw1 + gelu -> h1bf (gelu output, BN1 deferred); BN1 stats
        # interleaved per channel-chunk so the all-reduce fires early
        h1bf = bfp.tile([128, 4, BL, HW], BF16, tag="h1h2")
        h1f = h1bf.rearrange("p e b q -> p e (b q)")
        h1r = h1f.rearrange("p e (n q) -> p e n q", q=TCH)
        ybff = ybf.rearrange("p b q -> p (b q)")
        st1 = small.tile([128, 4, NT, 6], F32, tag="bnst")
        mv1 = small.tile([128, 4, 2], F32, tag="bnmv")
        for ec in range(4):
            for i in range(NT):
                pt = ps.tile([128, 2, 512], F32, tag="A" if i % 2 == 0 else "Bk")
                nc.tensor.matmul(pt[:, 0, 0:448], c1wT[:, ec * 128:(ec + 1) * 128],
                                 ybff[:, i * TCH:(i + 1) * TCH], start=True, stop=True)
                nc.scalar.activation(out=h1f[:, ec, i * TCH:(i + 1) * TCH], in_=pt[:, 0, 0:448],
                                     func=AF.Gelu, bias=c1_b[:, ec:ec + 1], scale=1.0)
            for i in range(NT):
                nc.vector.bn_stats(out=st1[:, ec, i], in_=h1r[:, ec, i])
            nc.vector.bn_aggr(out=mv1[:, ec], in_=st1[:, ec])
        gs1 = bn_pack_reduce(mv1, 4, ar_in[1], ar_out[1])
        a1, c1 = bn_affine(gs1, 4, bn1_g, bn1_b)
        # border constants: lh9 = psum9 * (c1/a1); eviction applies scale a1
        cra = small.tile([128, 4], F32, tag="cra")
        nc.vector.reciprocal(out=cra, in_=a1)
        nc.vector.tensor_mul(out=cra, in0=cra, in1=c1)
        nc.gpsimd.dma_start(out=c1_dram[:], in_=cra)
        c1row = small.tile([1, 4, 128], F32, tag="c1row")
        nc.gpsimd.dma_start(out=c1row, in_=bass.AP(tensor=c1_dram, offset=0, ap=[[0, 1], [1, 4], [4, 128]]))
        c1f = small.tile([9, 4, 128], F32, tag="c1f")
        nc.gpsimd.partition_broadcast(c1f, c1row)
        lh9 = small.tile([9, 4, 128], BF16, tag="lh9")
        nc.vector.tensor_mul(out=lh9, in0=psum9, in1=c1f)

        # FFN dw (raw taps; BN1 scale applied at eviction) + gelu -> h2g
        # software-pipelined: raw taps run DEPTH groups ahead of the
        # border matmul + eviction (which wait on the BN1 all-reduce)
        h2g = h1bf
        h2g4 = h2g.rearrange("p e b (h w) -> p e b h w", h=H)
        h1b4 = h1bf.rearrange("p e b (h w) -> p e b h w", h=H)
        h2f = h2g.rearrange("p e b q -> p e (b q)")
        st2 = small.tile([128, 4, BL, 2, 6], F32, tag="bnst2")
        mv2 = small.tile([128, 4, 2], F32, tag="bnmv2")
        tags4 = ["A", "Bk", "A2", "B2"]
        groups = [(ec, b) for ec in range(4) for b in range(BL)]
        DEPTH = 4
        pts = {}

        def conv_front(gidx):
            ec, b = groups[gidx]
            pt2 = ps.tile([128, 2, 512], F32, tag=tags4[gidx % 4], name="pt2")
            pts[gidx] = pt2
            for half in range(2):
                base = 14 * half
                pth = pt2[:, half, 0:392].rearrange("p (r c) -> p r c", c=W)
                dw3x3(pth, ffn_diag[:, ec], h1b4[:, ec, b], base)

        def conv_back(gidx):
            ec, b = groups[gidx]
            pt2 = pts.pop(gidx)
            for half in range(2):
                base = 14 * half
                pth = pt2[:, half, 0:392].rearrange("p (r c) -> p r c", c=W)
                nc.tensor.matmul(pth, lh9[:, ec], ind9[:, base:base + 14, :],
                                 start=False, stop=True, skip_group_check=True)
            for half in range(2):
                base = 14 * half
                pth = pt2[:, half, 0:392].rearrange("p (r c) -> p r c", c=W)
                nc.scalar.activation(out=h2g4[:, ec, b, base:base + 14, :], in_=pth,
                                     func=AF.Gelu, bias=dw_b[:, ec:ec + 1],
                                     scale=a1[:, ec:ec + 1])
            for half in range(2):
                nc.vector.bn_stats(out=st2[:, ec, b, half],
                                   in_=h2f[:, ec, b * HW + half * 392: b * HW + (half + 1) * 392])

        for gidx in range(len(groups)):
            conv_front(gidx)
            if gidx >= DEPTH - 1:
                conv_back(gidx - DEPTH + 1)
        for gidx in range(len(groups) - DEPTH + 1, len(groups)):
            conv_back(gidx)
        for ec in range(4):
            nc.vector.bn_aggr(out=mv2[:, ec], in_=st2[:, ec].rearrange("p b h s -> p (b h) s"))
        gs2 = bn_pack_reduce(mv2, 4, ar_in[2], ar_out[2])
        a2, c2 = bn_affine(gs2, 4, bnr_g, bnr_b)
        w2s = bfp.tile([128, 4, 128], BF16, tag="w2s")
        for kc in range(4):
            nc.vector.tensor_scalar(out=w2s[:, kc], in0=w2T[:, kc], scalar1=a2[:, kc:kc + 1],
                                    scalar2=None, op0=ALU.mult)
        ptb = ps.tile([128, 2, 512], F32, tag="A")
        for kc in range(4):
            nc.tensor.matmul(ptb[:, 0, 0:1], w2T[:, kc], c2[:, kc:kc + 1], start=(kc == 0), stop=(kc == 3))
        biasc = small.tile([128, 1], F32, tag="biascS")
        nc.vector.tensor_copy(out=biasc, in_=ptb[:, 0, 0:1])

        # pw2 -> h3s
        h3s = big.tile([128, BL, HW], F32, tag="big")
        h3f = h3s.rearrange("p b q -> p (b q)")
        h3r = h3f.rearrange("p (n q) -> p n q", q=TCH)
        st3 = small.tile([128, NT, 6], F32, tag="bnst3")
        mv3 = small.tile([128, 1, 2], F32, tag="bnmv3")
        for i in range(NT):
            pt = ps.tile([128, 2, 512], F32, tag="A" if i % 2 == 0 else "Bk")
            for kc in range(4):
                nc.tensor.matmul(pt[:, 0, 0:448], w2s[:, kc], h2f[:, kc, i * TCH:(i + 1) * TCH],
                                 start=(kc == 0), stop=(kc == 3))
            nc.vector.tensor_scalar(out=h3f[:, i * TCH:(i + 1) * TCH], in0=pt[:, 0, 0:448], scalar1=biasc,
                                    scalar2=None, op0=ALU.add)
            nc.vector.bn_stats(out=st3[:, i], in_=h3r[:, i])
        nc.vector.bn_aggr(out=mv3[:, 0], in_=st3)
        gs3 = bn_pack_reduce(mv3, 1, ar_in[3], ar_out[3])
        a3, c3 = bn_affine(gs3, 1, bn2_g, bn2_b)

        for b in range(BL):
            nc.vector.tensor_scalar(out=h3s[:, b], in0=h3s[:, b], scalar1=a3, scalar2=c3,
                                    op0=ALU.mult, op1=ALU.add)
            nc.vector.tensor_add(out=x_mhsa[:, b], in0=x_mhsa[:, b], in1=h3s[:, b])
            nc.sync.dma_start(out=out_t[b].rearrange("c h w -> c (h w)"), in_=x_mhsa[:, b])


_cached = None


def kernel(**inputs):
    global last_result, _cached
    hp = _host_prep(inputs)
    ln_triv = hp.pop("_ln_triv")
    if _cached is None or _cached[1] != ln_triv:
        _cached = (_build(ln_triv), ln_triv)
    nc = _cached[0]
    x = np.ascontiguousarray(np.asarray(inputs["x"], dtype=np.float32))
    in_maps = []
    for c in range(NC):
        m = dict(hp)
        m["xs"] = np.ascontiguousarray(x[c * BL:(c + 1) * BL])
        in_maps.append(m)
    trace = os.environ.get("KERNEL_TRACE", "0") == "1"
    res = run_bass_kernel_spmd(nc, in_maps, core_ids=list(range(NC)), trace=trace)
    last_result = res
    return np.concatenate([r["out"] for r in res.results], axis=0)
